# revision 16
# baseline (speedup 1.0000x reference)
"""Trainium2 Bass kernel for nn_ConflictAttentionModel.

Pure data parallel over batch B=512 across 8 NeuronCores (64 batches/core).
All weights replicated in SBUF. Per batch (A=128 agents, E=256, H=8 heads,
dh=32, L=2 layers):
  - gather selected city embeddings via indirect DMA
  - conflict mask built on-chip from acts
  - 2 transformer layers (masked MHA + FFN + LayerNorms)
  - final clipped single-head attention logits [A, A]

Residual stream kept natural [A(part), E(free)]; projections use PE
transposes. Attention scores = q_h @ k_h^T with the additive mask streamed
in as an identity-lhsT bias matmul accumulating into the same PSUM bank
first. Softmax denominators come from a ones-column appended to v in the
attn @ v matmul. Weight GEMMs run as float32r (1 cyc/row at N>=256),
attention internals in bf16, everything else fp32.
"""

import math
from contextlib import ExitStack

import numpy as np

B, A, N, E, H, HID, L = 512, 128, 1024, 256, 8, 512, 2
DH = E // H
NCORES = 8
NEG = -1e9
CLIP = 10.0
EPS = 1e-5
G = 4  # batches per projection group (N = G*A = 512)

_BUILD_CACHE = {}


def build_bass(bc, attn_bf16=True, use_f32r=True):
    import concourse.bacc as bacc
    import concourse.bass as bass
    import concourse.mybir as mybir
    import concourse.tile as tile
    from concourse.masks import make_identity

    f32 = mybir.dt.float32
    f32r = mybir.dt.float32r if use_f32r else mybir.dt.float32
    bf16 = mybir.dt.bfloat16 if attn_bf16 else mybir.dt.float32
    i32 = mybir.dt.int32
    AF = mybir.ActivationFunctionType
    OP = mybir.AluOpType
    assert bc % G == 0

    def r(ap):  # f32 -> f32r view for fast-matmul operands
        return ap.bitcast(f32r) if use_f32r else ap

    nc = bacc.Bacc("TRN2", target_bir_lowering=False, debug=False,
                   num_devices=NCORES)

    agent = nc.dram_tensor("agent", [bc, A, E], f32, kind="ExternalInput")
    cityflat = nc.dram_tensor("cityflat", [bc * N, E], f32, kind="ExternalInput")
    idxT_d = nc.dram_tensor("idxT", [A, bc], i32, kind="ExternalInput")
    actsT_d = nc.dram_tensor("actsT", [A, bc], f32, kind="ExternalInput")
    wspec = []
    for l in range(L):
        wspec += [(f"Wq{l}", E, E), (f"Wk{l}", E, E), (f"Wv{l}", E, E),
                  (f"Wo{l}", E, E), (f"W1{l}", E, HID), (f"W2{l}", HID, E)]
    wspec += [("Wqs", E, E), ("Wks", E, E)]
    wdram = {nm: nc.dram_tensor(nm, [ki, no], f32, kind="ExternalInput")
             for nm, ki, no in wspec}
    out_d = nc.dram_tensor("logits", [bc, A, A], f32, kind="ExternalOutput")

    with tile.TileContext(nc) as tc, ExitStack() as ctx:
        cpool = ctx.enter_context(tc.tile_pool(name="const", bufs=1))
        sb = ctx.enter_context(tc.tile_pool(name="sb", bufs=2))
        sb2 = ctx.enter_context(tc.tile_pool(name="sb2", bufs=2))
        pt = ctx.enter_context(tc.tile_pool(name="pt", bufs=2, space="PSUM"))
        pproj = ctx.enter_context(tc.tile_pool(name="pproj", bufs=2, space="PSUM"))
        patt = ctx.enter_context(tc.tile_pool(name="patt", bufs=2, space="PSUM"))
        pgen = ctx.enter_context(tc.tile_pool(name="pgen", bufs=2, space="PSUM"))

        # ---- constants ----
        ident = cpool.tile([A, A], f32, tag="ident")
        make_identity(nc, ident[:])
        ident_b = cpool.tile([A, A], bf16, tag="identb")
        nc.gpsimd.tensor_copy(out=ident_b[:], in_=ident[:])
        eps_t = cpool.tile([A, 1], f32, tag="epst")
        nc.gpsimd.memset(eps_t[:], EPS)

        wsb = {}
        for nm, ki, no in wspec:
            t = cpool.tile([A, ki // A, no], f32r, tag=nm)
            nc.sync.dma_start(out=t[:], in_=wdram[nm].ap().rearrange(
                "(kt p) n -> p kt n", p=A).bitcast(f32r))
            wsb[nm] = t
        idxT_s = cpool.tile([A, bc], i32, tag="idxT")
        nc.sync.dma_start(out=idxT_s[:], in_=idxT_d.ap())
        actsT_s = cpool.tile([A, bc], f32, tag="actsT")
        nc.sync.dma_start(out=actsT_s[:], in_=actsT_d.ap())

        def layer_norm(x, out, j):
            bn6 = sb2.tile([A, 6], f32, tag=f"bn6_{j}")
            nc.vector.bn_stats(out=bn6[:], in_=x[:])
            mv = sb2.tile([A, 2], f32, tag=f"mv_{j}")
            nc.vector.bn_aggr(out=mv[:], in_=bn6[:])
            std = sb2.tile([A, 1], f32, tag=f"std_{j}")
            nc.scalar.activation(out=std[:], in_=mv[:, 1:2], func=AF.Sqrt,
                                 bias=eps_t[:, 0:1])
            rstd = sb2.tile([A, 1], f32, tag=f"rstd_{j}")
            nc.vector.reciprocal(out=rstd[:], in_=std[:])
            nc.vector.tensor_scalar(out=out[:], in0=x[:], scalar1=mv[:, 0:1],
                                    scalar2=rstd[:, 0:1], op0=OP.subtract,
                                    op1=OP.mult)

        def transpose_to(dst_ap, x_ap, ktiles):
            """transpose natural [A, ktiles*128] -> dst sbuf AP [128, ktiles, A]."""
            ps = pt.tile([A, 4, A], f32, space="PSUM", tag="pt")
            for k in range(ktiles):
                nc.tensor.transpose(out=ps[:, k, :],
                                    in_=x_ap[:, k * A:(k + 1) * A],
                                    identity=ident[:])
            nc.scalar.copy(out=dst_ap, in_=ps[:, 0:ktiles, :])

        def transpose_nat(x_ap, ktiles, tag):
            res = sb.tile([A, ktiles, A], f32r, tag=tag)
            transpose_to(res[:], x_ap, ktiles)
            return res

        def proj_T(w, rhsT, tag, dtype):
            """out[128, 2, G*A] = W^T @ xT for a group of G batches."""
            res = sb.tile([A, 2, G * A], dtype, tag=tag)
            for mt in range(2):
                ps = pproj.tile([A, G * A], f32, space="PSUM", tag="pproj")
                for kt in range(2):
                    nc.tensor.matmul(
                        out=ps[:], lhsT=wsb[w][:, kt, mt * A:(mt + 1) * A],
                        rhs=rhsT[:, kt, :], start=(kt == 0), stop=(kt == 1))
                nc.scalar.copy(out=res[:, mt, :], in_=ps[:])
            return res

        def proj_T_heads(w, rhsT, tag, dtype):
            """Head-friendly projection: out[0:64, t, :] holds features
            t*64..t*64+63 (head pair 2t, 2t+1), so every 32-row head slice
            starts at partition 0 or 32 (matmul base-partition rule)."""
            res = sb.tile([A, 4, G * A], dtype, tag=tag)
            for mt in range(4):
                ps = pproj.tile([A, G * A], f32, space="PSUM", tag="pproj")
                for kt in range(2):
                    nc.tensor.matmul(
                        out=ps[0:64, :],
                        lhsT=wsb[w][:, kt, mt * 64:(mt + 1) * 64],
                        rhs=rhsT[:, kt, :], start=(kt == 0), stop=(kt == 1))
                nc.scalar.copy(out=res[0:64, mt, :], in_=ps[0:64, :])
            return res

        for grp in range(bc // G):
            bs = [grp * G + j for j in range(G)]

            # ---- loads, input transposes, conflict masks ----
            kv_nat, cac_nat, maskb_f, maskb_b = {}, {}, {}, {}
            kvT_g = sb.tile([A, 2, G * A], f32r, tag="kvT")
            cacT_g = sb.tile([A, 2, G * A], f32r, tag="cacT")
            for j, b in enumerate(bs):
                ja = slice(j * A, (j + 1) * A)
                kv = sb.tile([A, E], f32, tag=f"kv{j}")
                nc.sync.dma_start(out=kv[:], in_=agent.ap()[b])
                kv_nat[j] = kv
                cac = sb.tile([A, E], f32, tag=f"cac{j}")
                nc.gpsimd.indirect_dma_start(
                    out=cac[:], out_offset=None, in_=cityflat.ap(),
                    in_offset=bass.IndirectOffsetOnAxis(
                        ap=idxT_s[:, b:b + 1], axis=0))
                cac_nat[j] = cac
                transpose_to(kvT_g[:, :, ja], kv[:], 2)
                transpose_to(cacT_g[:, :, ja], cac[:], 2)

                # conflict mask (gpsimd, off the hot engines)
                acol = actsT_s[:, b:b + 1]
                psa = pt.tile([A, 4, A], f32, space="PSUM", tag="pt")
                nc.tensor.transpose(out=psa[:, 0, :],
                                    in_=acol.to_broadcast([A, A]),
                                    identity=ident[:])
                arow = sb2.tile([A, A], f32, tag=f"arow{j}")
                nc.scalar.copy(out=arow[:], in_=psa[:, 0, :])
                eq = sb2.tile([A, A], f32, tag=f"eq{j}")
                nc.vector.tensor_tensor(out=eq[:], in0=acol.to_broadcast([A, A]),
                                        in1=arow[:], op=OP.is_equal)
                sel = sb2.tile([A, 1], f32, tag=f"sel{j}")
                nc.gpsimd.tensor_scalar(out=sel[:], in0=acol, scalar1=0.0,
                                        scalar2=None, op0=OP.is_equal)
                dif = sb2.tile([A, A], f32, tag=f"dif{j}")
                nc.gpsimd.tensor_tensor(out=dif[:], in0=ident[:], in1=eq[:],
                                        op=OP.subtract)
                selfix = sb2.tile([A, A], f32, tag=f"selfix{j}")
                nc.gpsimd.tensor_scalar(out=selfix[:], in0=dif[:],
                                        scalar1=sel[:, 0:1], scalar2=None,
                                        op0=OP.mult)
                conf = sb2.tile([A, A], f32, tag=f"conf{j}")
                nc.gpsimd.tensor_tensor(out=conf[:], in0=eq[:], in1=selfix[:],
                                        op=OP.add)
                mbf = sb2.tile([A, A], f32, tag=f"mbf{j}")
                nc.gpsimd.tensor_scalar(out=mbf[:], in0=conf[:], scalar1=1e9,
                                        scalar2=NEG, op0=OP.mult, op1=OP.add)
                maskb_f[j] = mbf
                mbb = sb2.tile([A, A], bf16, tag=f"mbb{j}")
                nc.gpsimd.tensor_copy(out=mbb[:], in_=mbf[:])
                maskb_b[j] = mbb

            # ---- transformer layers ----
            x_nat = cac_nat
            xT_g = cacT_g
            for l in range(L):
                qT = proj_T_heads(f"Wq{l}", xT_g, "qT", bf16)
                kT = proj_T_heads(f"Wk{l}", kvT_g, "kT", bf16)
                x_next = {}
                xT_next = sb.tile([A, 2, G * A], f32r, tag="xTn", name="xTn") if l + 1 < L else None
                for j in range(G):
                    ja = slice(j * A, (j + 1) * A)
                    # v projection (natural) with ones column appended
                    psv = pgen.tile([A, HID], f32, space="PSUM", tag="pgen")
                    for kt in range(2):
                        nc.tensor.matmul(out=psv[:, 0:E],
                                         lhsT=kvT_g[:, kt, ja],
                                         rhs=wsb[f"Wv{l}"][:, kt, :],
                                         start=(kt == 0), stop=(kt == 1))
                    v_aug = sb.tile([A, H, DH + 1], bf16, tag="vaug")
                    nc.gpsimd.memset(v_aug[:], 1.0)
                    nc.scalar.copy(
                        out=v_aug[:, :, 0:DH],
                        in_=psv[:, 0:E].rearrange("p (h d) -> p h d", h=H))

                    # attention: two 4-head PSUM banks. All matmuls into one
                    # bank must share a PE row-tile base (different row-tiles
                    # writing one PSUM bank is a HW fault), so bank g holds
                    # heads with h % 2 == g — their data sits at partition
                    # offset 32*g in the proj_T_heads layout.
                    exp_s = sb.tile([A, H, A], bf16, tag="exps")
                    for g in range(2):
                        off = 32 * g
                        ps_s = patt.tile([A, 4, A], f32, space="PSUM", tag="patt")
                        nc.tensor.matmul(
                            out=ps_s[:], lhsT=ident_b[:],
                            rhs=maskb_b[j][:, None, :].to_broadcast([A, 4, A]),
                            start=True, stop=False)
                        for hh in range(4):
                            nc.tensor.matmul(
                                out=ps_s[:, hh, :],
                                lhsT=kT[off:off + 32, hh, ja],
                                rhs=qT[off:off + 32, hh, ja],
                                start=False, stop=(hh == 3))
                        nc.scalar.activation(out=exp_s[:, 4 * g:4 * (g + 1), :],
                                             in_=ps_s[:], func=AF.Exp)
                    # exp_s slot 4*g + hh holds head h = 2*hh + g.
                    # exp_s is exp(scores)^T = [k, q]; contraction over k is
                    # exactly what attn @ v needs, no transpose required.
                    ps_av = pgen.tile([A, H, DH + 1], f32, space="PSUM", tag="pgen")
                    for g in range(2):
                        for hh in range(4):
                            h = 2 * hh + g
                            nc.tensor.matmul(out=ps_av[:, h, :],
                                             lhsT=exp_s[:, 4 * g + hh, :],
                                             rhs=v_aug[:, h, :],
                                             start=True, stop=True)
                    recip = sb.tile([A, H], f32, tag="recip")
                    nc.vector.reciprocal(out=recip[:], in_=ps_av[:, :, DH])
                    att_o = sb.tile([A, E], f32, tag="atto")
                    nc.vector.tensor_tensor(
                        out=att_o[:].rearrange("p (h d) -> p h d", h=H),
                        in0=ps_av[:, :, 0:DH],
                        in1=recip[:, :, None].to_broadcast([A, H, DH]),
                        op=OP.mult)
                    attT = transpose_nat(att_o[:], 2, "attT")

                    # y = att_o @ Wo ; r1 = x + y ; LN1
                    psy = pgen.tile([A, HID], f32, space="PSUM", tag="pgen")
                    for kt in range(2):
                        nc.tensor.matmul(out=psy[:, 0:E], lhsT=attT[:, kt, :],
                                         rhs=wsb[f"Wo{l}"][:, kt, :],
                                         start=(kt == 0), stop=(kt == 1))
                    r1 = sb.tile([A, E], f32, tag="r1")
                    nc.vector.tensor_tensor(out=r1[:], in0=psy[:, 0:E],
                                            in1=x_nat[j][:], op=OP.add)
                    h_nat = sb.tile([A, E], f32, tag="hnat")
                    layer_norm(r1, h_nat, j)

                    # FFN
                    hT = transpose_nat(h_nat[:], 2, "hT")
                    psf = pgen.tile([A, HID], f32, space="PSUM", tag="pgen")
                    for kt in range(2):
                        nc.tensor.matmul(out=psf[:], lhsT=hT[:, kt, :],
                                         rhs=wsb[f"W1{l}"][:, kt, :],
                                         start=(kt == 0), stop=(kt == 1))
                    f_nat = sb.tile([A, HID], f32, tag="fnat")
                    nc.scalar.activation(out=f_nat[:], in_=psf[:], func=AF.Relu)
                    fT = transpose_nat(f_nat[:], 4, "fT")
                    ps2 = pgen.tile([A, HID], f32, space="PSUM", tag="pgen")
                    for kt in range(4):
                        nc.tensor.matmul(out=ps2[:, 0:E], lhsT=fT[:, kt, :],
                                         rhs=wsb[f"W2{l}"][:, kt, :],
                                         start=(kt == 0), stop=(kt == 3))
                    r2 = sb.tile([A, E], f32, tag="r2")
                    nc.vector.tensor_tensor(out=r2[:], in0=ps2[:, 0:E],
                                            in1=h_nat[:], op=OP.add)
                    xn = sb.tile([A, E], f32, tag=f"xn{j}")
                    layer_norm(r2, xn, j)
                    x_next[j] = xn
                    if xT_next is not None:
                        transpose_to(xT_next[:, :, ja], xn[:], 2)
                x_nat = x_next
                if xT_next is not None:
                    xT_g = xT_next

            # ---- final single-head clipped attention ----
            x2T_g = sb.tile([A, 2, G * A], f32r, tag="x2T")
            for j in range(G):
                transpose_to(x2T_g[:, :, j * A:(j + 1) * A], x_nat[j][:], 2)
            qsT = proj_T("Wqs", x2T_g, "qsT", f32)
            ksT = proj_T("Wks", kvT_g, "ksT", f32)
            for j in range(G):
                ja = slice(j * A, (j + 1) * A)
                psl = pgen.tile([A, HID], f32, space="PSUM", tag="pgen")
                for kt in range(2):
                    nc.tensor.matmul(out=psl[:, 0:A], lhsT=qsT[:, kt, ja],
                                     rhs=ksT[:, kt, ja],
                                     start=(kt == 0), stop=(kt == 1))
                th = sb.tile([A, A], f32, tag="th")
                nc.scalar.activation(out=th[:], in_=psl[:, 0:A], func=AF.Tanh)
                th10 = sb.tile([A, A], f32, tag="th10")
                nc.vector.tensor_scalar(out=th10[:], in0=th[:], scalar1=CLIP,
                                        scalar2=None, op0=OP.mult)
                lg = sb.tile([A, A], f32, tag="lg")
                nc.vector.tensor_tensor(out=lg[:], in0=th10[:],
                                        in1=maskb_f[j][:], op=OP.add)
                nc.sync.dma_start(out=out_d.ap()[bs[j]], in_=lg[:])

    nc.compile()
    return nc


def host_prepare(agent_embed, city_embed, acts, params, bc):
    """Split full inputs into per-core in_maps. Folds 1/sqrt(d) into Wq."""
    agent_embed = np.ascontiguousarray(np.asarray(agent_embed, np.float32))
    city_embed = np.ascontiguousarray(np.asarray(city_embed, np.float32))
    acts = np.asarray(acts).astype(np.int64)
    ncores = agent_embed.shape[0] // bc

    weights = {}
    for l, p in enumerate(params["layers"]):
        for k in ("b1", "b2", "ln1_b", "ln2_b"):
            assert np.abs(np.asarray(p[k])).max() == 0.0, f"nonzero {k}"
        for k in ("ln1_s", "ln2_s"):
            assert np.abs(np.asarray(p[k]) - 1.0).max() == 0.0, f"nontrivial {k}"
        weights[f"Wq{l}"] = np.asarray(p["Wq"], np.float32) / math.sqrt(DH)
        weights[f"Wk{l}"] = np.asarray(p["Wk"], np.float32)
        weights[f"Wv{l}"] = np.asarray(p["Wv"], np.float32)
        weights[f"Wo{l}"] = np.asarray(p["Wo"], np.float32)
        weights[f"W1{l}"] = np.asarray(p["W1"], np.float32)
        weights[f"W2{l}"] = np.asarray(p["W2"], np.float32)
    weights["Wqs"] = np.asarray(params["Wq_s"], np.float32) / math.sqrt(E)
    weights["Wks"] = np.asarray(params["Wk_s"], np.float32)
    weights = {k: np.ascontiguousarray(v) for k, v in weights.items()}

    in_maps = []
    boff = np.arange(bc, dtype=np.int64)[:, None] * N
    for c in range(ncores):
        sl = slice(c * bc, (c + 1) * bc)
        acts_sh = acts[sl]
        m = dict(weights)
        m["agent"] = agent_embed[sl]
        m["cityflat"] = np.ascontiguousarray(
            city_embed[sl].reshape(bc * N, E))
        m["idxT"] = np.ascontiguousarray((acts_sh + boff).T.astype(np.int32))
        m["actsT"] = np.ascontiguousarray(acts_sh.T.astype(np.float32))
        in_maps.append(m)
    return in_maps


def kernel(agent_embed, city_embed, acts, params):
    from concourse.bass_utils import run_bass_kernel_spmd

    bc = np.asarray(agent_embed).shape[0] // NCORES
    key = ("main", bc)
    if key not in _BUILD_CACHE:
        nc = build_bass(bc)
        nc.finalize()
        _BUILD_CACHE[key] = nc
    nc = _BUILD_CACHE[key]
    in_maps = host_prepare(agent_embed, city_embed, acts, params, bc)
    res = run_bass_kernel_spmd(nc, in_maps, list(range(NCORES)))
    return np.concatenate([res.results[c]["logits"] for c in range(NCORES)],
                          axis=0)
